# revision 6
# baseline (speedup 1.0000x reference)
"""Trainium2 Bass kernel for nn_AtlasDetection (voxel->pixel feature gather).

kernel(**inputs) takes the FULL inputs and returns (volume, valid) exactly
like the reference. Internally it shards the voxel grid along nx across 8
NeuronCores, runs a Bass/Tile kernel per core, and concatenates the slabs.

Device pipeline per core (separable-camera fast path):
  stage A: gpsimd ap_gather x-gathers image columns px(gx,gz) from an SBUF
           table [h, (w, c)] (bf16), amortizing the per-index cost over the
           32-channel payload -> A[h, (gz, gx, c)]
  stage B: PE one-hot matmul Sel[h, gy] expands+transposes to
           [(gx, c), gy] per gz plane (PSUM)
  drain:   ACT/DVE copy PSUM -> SBUF accumulator with a gz-strided access
           pattern, building [(gx, c), (gy, gz)] runs
  write:   contiguous 40KB-per-partition DMA to volume[b, c, gx, :, :]
Invalid voxels read a zero row/column of the table (sentinel index), so no
mask multiply is needed. The tiny per-plane index math (2*64*(160+20)
values) runs on host; all O(N)-scale data movement happens on device.
"""

import subprocess

import numpy as np
import ml_dtypes
import orjson

import concourse.bass as bass  # noqa: F401
import concourse.mybir as mybir
from concourse import bacc, bass2jax, bass_utils
from concourse.tile import TileContext

# ---------------------------------------------------------------------------
# Container compile fixups:
#  - the walrus in this image supports ONE sync-wait per instruction; split
#    extras onto same-engine NoOps (program order makes this equivalent)
#  - use the b16 walrus build (knows the custom ISA instructions)
# ---------------------------------------------------------------------------
_split_counter = [0]


def _split_multi_waits(bir: dict) -> int:
    n_split = 0
    for func in bir.get("functions", []):
        for blk in func.get("blocks", []):
            insts = blk.get("instructions")
            if not insts:
                continue
            out = []
            for ins in insts:
                si = ins.get("sync_info")
                waits = si.get("on_wait") if si else None
                if waits and len(waits) > 1:
                    keep = waits[-1]
                    for w in waits[:-1]:
                        _split_counter[0] += 1
                        out.append(
                            {
                                "name": f"antsplitw-{_split_counter[0]}",
                                "opcode": "NoOp",
                                "engine": ins["engine"],
                                "ins": [],
                                "outs": [],
                                "sync_info": {"on_wait": [w], "on_update": []},
                            }
                        )
                    si["on_wait"] = [keep]
                    n_split += 1
                out.append(ins)
            blk["instructions"] = out
    return n_split


if not getattr(bass_utils, "_atlas_patched", False):
    bass_utils._atlas_patched = True
    _orig_compile_bir_kernel = bass_utils.compile_bir_kernel

    def _patched_compile_bir_kernel(bir_json, tmpdir, neff_name="file.neff"):
        bir = orjson.loads(bir_json)
        if _split_multi_waits(bir):
            bir_json = orjson.dumps(bir)
        try:
            return _orig_compile_bir_kernel(bir_json, tmpdir, neff_name)
        except subprocess.CalledProcessError as e:
            out = e.stdout
            if isinstance(out, bytes):
                out = out.decode(errors="replace")
            lines = (out or "").splitlines()
            tail = [l for l in lines if "ERROR" in l or "Too many" in l]
            print("==== WALRUS COMPILE FAILURE ====")
            print("\n".join(tail[-8:]) if tail else "\n".join(lines[-25:]))
            print("================================", flush=True)
            raise

    bass_utils.compile_bir_kernel = _patched_compile_bir_kernel
    bass2jax.compile_bir_kernel = _patched_compile_bir_kernel

    import glob as _glob

    _b16 = _glob.glob(
        "/nix/store/*b16-bazel*/lib/python3.13/site-packages/neuronxcc/"
        "starfish/bin/walrus_driver"
    )
    if _b16:
        bass_utils.get_walrus_driver = lambda: _b16[0]


# ---------------------------------------------------------------------------
# Problem constants (hardcoded per spec)
# ---------------------------------------------------------------------------
NX, NY, NZ = 160, 160, 64
VOXEL_SIZE = np.float32(0.04)
STRIDE = 4.0
B, C, H, W = 2, 32, 120, 160
N_CORES = 8
GXC = NX // N_CORES        # 20 gx per core
NQ, GXQ = 5, 4             # gx chunks per core
NYZ = NY * NZ              # 10240
HP = H + 1                 # 121 rows incl zero-sentinel row
WP = W + 1                 # 161 cols incl zero-sentinel col
TBL_F = WP * C             # 5152 table free elems per h-row
BF16 = mybir.dt.bfloat16
F32 = mybir.dt.float32
I16 = mybir.dt.int16
U8 = mybir.dt.uint8


def _build_nc():
    """Build the 8-core SPMD Tile kernel graph (finalized Bacc)."""
    nc = bacc.Bacc(
        "TRN2", target_bir_lowering=False, debug=False, num_devices=N_CORES
    )
    table_d = nc.declare_dram_parameter("table", [B, 128, TBL_F], BF16, isOutput=False)
    pxw_d = nc.declare_dram_parameter("pxw", [B, NQ, 128, 16], I16, isOutput=False)
    pyw_d = nc.declare_dram_parameter("pyw", [B, NZ, NY], F32, isOutput=False)
    vx_d = nc.declare_dram_parameter("vx", [B, GXC, NZ], U8, isOutput=False)
    vy_d = nc.declare_dram_parameter("vy", [B, NY, NZ], U8, isOutput=False)
    vol_d = nc.declare_dram_parameter(
        "vol", [B, C, GXC, NY, NZ], F32, isOutput=True
    )
    valid_d = nc.declare_dram_parameter(
        "valid", [B, GXC, NY, NZ], U8, isOutput=True
    )

    with TileContext(nc) as tc:
        with (
            tc.tile_pool(name="const", bufs=1) as constp,
            tc.tile_pool(name="selp", bufs=2) as selp,
            tc.tile_pool(name="pybc", bufs=4) as pybc,
            tc.tile_pool(name="idxp", bufs=4) as idxp,
            tc.tile_pool(name="apool", bufs=2) as apool,
            tc.tile_pool(name="accp", bufs=2) as accp,
            tc.tile_pool(name="vldp", bufs=2) as vldp,
            tc.tile_pool(name="psum", bufs=4, space="PSUM") as psump,
        ):
            iota_t = constp.tile([128, 1], F32)
            nc.gpsimd.iota(
                iota_t[:],
                pattern=[[0, 1]],
                base=0,
                channel_multiplier=1,
                allow_small_or_imprecise_dtypes=True,
            )

            tabs = []
            for b in range(B):
                tb = constp.tile([128, TBL_F], BF16, tag="tab")
                nc.sync.dma_start(out=tb[:], in_=table_d[b])
                tabs.append(tb)

            sels = []
            for b in range(B):
                # one-hot Sel[h, gy] per gz, packed [128, 64*160] bf16
                sel = selp.tile([128, NZ * NY], BF16)
                for gz in range(NZ):
                    pyt = pybc.tile([128, NY], F32)
                    nc.sync.dma_start(
                        out=pyt[:], in_=pyw_d[b, gz, :].partition_broadcast(128)
                    )
                    nc.vector.tensor_scalar(
                        out=sel[:, gz * NY : (gz + 1) * NY],
                        in0=pyt[:],
                        scalar1=iota_t[:],
                        scalar2=None,
                        op0=mybir.AluOpType.is_equal,
                    )
                sels.append(sel)

                # valid[b] = vx[gx, gz] & vy[gy, gz] -> [20, gy, gz]
                vyt = vldp.tile([GXC, NY, NZ], U8, tag="vy")
                nc.sync.dma_start(
                    out=vyt[:], in_=vy_d[b].partition_broadcast(GXC)
                )
                vxt = vldp.tile([GXC, 1, NZ], U8, tag="vx")
                nc.sync.dma_start(out=vxt[:, 0, :], in_=vx_d[b])
                vout = vldp.tile([GXC, NY, NZ], U8, tag="vout")
                nc.vector.tensor_tensor(
                    out=vout[:],
                    in0=vyt[:],
                    in1=vxt[:].broadcast_to((GXC, NY, NZ)),
                    op=mybir.AluOpType.mult,
                )
                nc.sync.dma_start(out=valid_d[b], in_=vout[:])

            for b in range(B):
                for q in range(NQ):
                    it = idxp.tile([128, 16], I16)
                    nc.sync.dma_start(out=it[:], in_=pxw_d[b, q])
                    aq = apool.tile([128, NZ * GXQ * C], BF16)  # [h,(gz,gx,c)]
                    nc.gpsimd.ap_gather(
                        out_ap=aq[:],
                        in_ap=tabs[b][:],
                        idxs_ap=it[:],
                        channels=128,
                        num_elems=WP,
                        d=C,
                        num_idxs=NZ * GXQ,
                    )
                    acc = accp.tile([128, NYZ], F32)
                    acc3 = acc[:].rearrange("p (gy gz) -> p gy gz", gz=NZ)
                    for i3, g0 in enumerate(range(0, NZ, 3)):
                        n3 = min(3, NZ - g0)
                        ps = psump.tile([128, 480], F32)
                        for j in range(n3):
                            gz = g0 + j
                            nc.tensor.matmul(
                                ps[:, j * NY : (j + 1) * NY],
                                aq[:, gz * 128 : (gz + 1) * 128],
                                sels[b][:, gz * NY : (gz + 1) * NY],
                                start=True,
                                stop=True,
                            )
                        src = ps[:, : n3 * NY].rearrange("p (j gy) -> p j gy", j=n3)
                        dst = acc3[:, :, g0 : g0 + n3].rearrange("p gy j -> p j gy")
                        if i3 % 2 == 0:
                            nc.vector.tensor_copy(dst, src)
                        else:
                            nc.scalar.copy(dst, src)
                    nc.sync.dma_start(
                        out=vol_d[b, :, q * GXQ : (q + 1) * GXQ].rearrange(
                            "c g gy gz -> g c (gy gz)"
                        ),
                        in_=acc[:],
                    )
    nc.finalize()
    return nc


# ---------------------------------------------------------------------------
# Host-side prep: per-plane projection math (exact f32, matching reference)
# ---------------------------------------------------------------------------
def _host_prep(projection, features, origin):
    proj = projection.astype(np.float32) * np.array(
        [1.0 / STRIDE, 1.0 / STRIDE, 1.0], np.float32
    )[None, :, None]
    sep = (
        np.all(proj[:, 0, 1] == 0.0)
        and np.all(proj[:, 1, 0] == 0.0)
        and np.all(proj[:, 2, 0] == 0.0)
        and np.all(proj[:, 2, 1] == 0.0)
    )
    if not sep:
        return None

    # Plane-index math on jax-CPU with the reference's exact op sequence, so
    # exact-tie roundings (e.g. py = 1.25*gy - 40 hitting .5) match bitwise.
    # Coefficients of the "other" grid axis are exactly 0 (separability), so
    # evaluating the einsum on a sub-grid reproduces the same floats.
    import jax
    import jax.numpy as jnp

    with jax.default_device(jax.devices("cpu")[0]):
        ogj = jnp.asarray(origin.astype(np.float32))
        projj = jnp.asarray(proj)

        def cam_grid(axis_n, axis_idx):
            # sub-grid over (axis, gz); other spatial coord pinned to 0
            ga = jnp.arange(axis_n)
            gzv = jnp.arange(NZ)
            a, z = jnp.meshgrid(ga, gzv, indexing="ij")
            zero = jnp.zeros_like(a.ravel())
            cols = [zero, zero, z.ravel()]
            cols[axis_idx] = a.ravel()
            coords = jnp.stack(cols).astype(np.float32)
            world = coords[None] * float(VOXEL_SIZE) + ogj[:, :, None]
            n = axis_n * NZ
            world = jnp.concatenate(
                [jnp.broadcast_to(world, (B, 3, n)), jnp.ones((B, 1, n), np.float32)],
                axis=1,
            )
            camera = jnp.einsum("bij,bjn->bin", projj, world)
            pz = camera[:, 2, :]
            p0 = jnp.round(camera[:, 0, :] / pz)
            p1 = jnp.round(camera[:, 1, :] / pz)
            return (
                np.asarray(p0).reshape(B, axis_n, NZ),
                np.asarray(p1).reshape(B, axis_n, NZ),
                np.asarray(pz).reshape(B, axis_n, NZ),
            )

        pxf, _, pzx = cam_grid(NX, 0)   # [B, gx, gz]
        _, pyf, pzy = cam_grid(NY, 1)   # [B, gy, gz]

    with np.errstate(invalid="ignore"):
        vx = (pxf >= 0) & (pxf < W) & (pzx > 0)          # [B, gx, gz]
        vy = (pyf >= 0) & (pyf < H) & (pzy > 0)          # [B, gy, gz]
    pxg = np.where(vx, pxf, np.float32(W)).astype(np.int16).transpose(0, 2, 1)
    pyg = np.where(vy, pyf, np.float32(H)).astype(np.float32).transpose(0, 2, 1)
    vxg = vx.astype(np.uint8)
    vyg = vy.astype(np.uint8)

    # bf16 feature table [b, h(128 padded), (w(161), c)], sentinels zero
    table = np.zeros((B, 128, WP, C), np.float32)
    for b in range(B):
        table[b, :H, :W, :] = features[b].transpose(1, 2, 0)
    table = table.reshape(B, 128, TBL_F).astype(ml_dtypes.bfloat16)

    in_maps = []
    for core in range(N_CORES):
        s = core * GXC
        pxw = np.empty((B, NQ, 128, 16), np.int16)
        for b in range(B):
            for q in range(NQ):
                flat = pxg[b][:, s + q * GXQ : s + (q + 1) * GXQ].reshape(-1)
                wrapped = np.ascontiguousarray(flat.reshape(16, 16).T)
                pxw[b, q] = np.tile(wrapped, (8, 1))
        in_maps.append(
            {
                "table": table,
                "pxw": pxw,
                "pyw": pyg,
                "vx": np.ascontiguousarray(vxg[:, s : s + GXC, :]),
                "vy": vyg,
            }
        )
    return in_maps


# ---------------------------------------------------------------------------
# Persistent SPMD runner (compile once, rebind inputs per call)
# ---------------------------------------------------------------------------
class _Runner:
    def __init__(self, nc):
        import jax
        from jax.sharding import Mesh, PartitionSpec, NamedSharding
        from jax.experimental.shard_map import shard_map
        from concourse.bass2jax import (
            _bass_exec_p,
            install_neuronx_cc_hook,
            partition_id_tensor,
        )

        install_neuronx_cc_hook()
        self.jax = jax
        partition_name = nc.partition_id_tensor.name if nc.partition_id_tensor else None
        in_names, out_names, out_avals, zero_outs = [], [], [], []
        for alloc in nc.m.functions[0].allocations:
            if not isinstance(alloc, mybir.MemoryLocationSet):
                continue
            name = alloc.memorylocations[0].name
            if alloc.kind == "ExternalInput":
                if name != partition_name:
                    in_names.append(name)
            elif alloc.kind == "ExternalOutput":
                shape = tuple(alloc.tensor_shape)
                dtype = mybir.dt.np(alloc.dtype)
                out_names.append(name)
                out_avals.append(jax.core.ShapedArray(shape, dtype))
                zero_outs.append(np.zeros(shape, dtype))
        n_params, n_outs = len(in_names), len(out_avals)
        all_in = list(in_names) + list(out_names)
        if partition_name is not None:
            all_in.append(partition_name)
        self.in_names, self.out_names, self.out_avals = in_names, out_names, out_avals

        def _body(*args):
            operands = list(args)
            if partition_name is not None:
                operands.append(partition_id_tensor())
            return tuple(
                _bass_exec_p.bind(
                    *operands,
                    out_avals=tuple(out_avals),
                    in_names=tuple(all_in),
                    out_names=tuple(out_names),
                    lowering_input_output_aliases=(),
                    sim_require_finite=True,
                    sim_require_nnan=True,
                    nc=nc,
                )
            )

        devices = jax.devices()[:N_CORES]
        mesh = Mesh(np.asarray(devices), ("core",))
        self._sharding = NamedSharding(mesh, PartitionSpec("core"))
        self._fn = jax.jit(
            shard_map(
                _body,
                mesh=mesh,
                in_specs=(PartitionSpec("core"),) * (n_params + n_outs),
                out_specs=(PartitionSpec("core"),) * n_outs,
                check_rep=False,
            ),
            keep_unused=True,
        )
        self._zero_outs = zero_outs
        self._dev_zeros = [
            jax.device_put(
                np.zeros((N_CORES * z.shape[0], *z.shape[1:]), z.dtype),
                self._sharding,
            )
            for z in zero_outs
        ]

    def set_inputs(self, in_maps):
        self._dev_in = [
            self.jax.device_put(
                np.concatenate(
                    [np.asarray(in_maps[c][k]) for c in range(N_CORES)], axis=0
                ),
                self._sharding,
            )
            for k in self.in_names
        ] + self._dev_zeros

    def run_device(self):
        outs = self._fn(*self._dev_in)
        for o in outs:
            o.block_until_ready()
        return outs

    def run(self):
        outs = self.run_device()
        res = {}
        for i, name in enumerate(self.out_names):
            a = np.asarray(outs[i])
            res[name] = a.reshape(N_CORES, *self.out_avals[i].shape)
        return res


_CACHE = {}


def _get_runner():
    if "runner" not in _CACHE:
        _CACHE["runner"] = _Runner(_build_nc())
    return _CACHE["runner"]


def _reference_fallback(projection, features, origin):
    """Numpy fallback for non-separable projections (not hit by the
    benchmark's camera; kept for robustness)."""
    proj = projection.astype(np.float32) * np.array(
        [1.0 / STRIDE, 1.0 / STRIDE, 1.0], np.float32
    )[None, :, None]
    gx, gy, gz = np.meshgrid(
        np.arange(NX), np.arange(NY), np.arange(NZ), indexing="ij"
    )
    coords = np.stack([gx.ravel(), gy.ravel(), gz.ravel()]).astype(np.float32)
    world = coords[None] * VOXEL_SIZE + origin.astype(np.float32)[:, :, None]
    N = NX * NY * NZ
    world = np.concatenate(
        [np.broadcast_to(world, (B, 3, N)), np.ones((B, 1, N), np.float32)], axis=1
    )
    camera = np.einsum("bij,bjn->bin", proj, world)
    pz = camera[:, 2, :]
    with np.errstate(divide="ignore", invalid="ignore", over="ignore"):
        px = np.rint(camera[:, 0, :] / pz)
        py = np.rint(camera[:, 1, :] / pz)
        valid = (px >= 0) & (py >= 0) & (px < W) & (py < H) & (pz > 0)
        pxi = np.clip(px, 0, W - 1).astype(np.int32)
        pyi = np.clip(py, 0, H - 1).astype(np.int32)
    idx = pyi * W + pxi
    feat_flat = features.reshape(B, C, H * W)
    vol = np.take_along_axis(feat_flat, idx[:, None, :], axis=2)
    vol = vol * valid[:, None, :].astype(features.dtype)
    return (
        vol.reshape(B, C, NX, NY, NZ),
        valid.reshape(B, 1, NX, NY, NZ),
    )


def kernel(projection, features, origin):
    projection = np.asarray(projection)
    features = np.asarray(features, dtype=np.float32)
    origin = np.asarray(origin)
    in_maps = _host_prep(projection, features, origin)
    if in_maps is None:
        return _reference_fallback(projection, features, origin)
    r = _get_runner()
    r.set_inputs(in_maps)
    res = r.run()
    vol = np.concatenate([res["vol"][c] for c in range(N_CORES)], axis=2)
    valid = np.concatenate([res["valid"][c] for c in range(N_CORES)], axis=1)
    return vol, valid.astype(bool).reshape(B, 1, NX, NY, NZ)


# revision 9
# speedup vs baseline: 2.7400x; 2.7400x over previous
"""Trainium2 Bass kernel for nn_AtlasDetection (voxel->pixel feature gather).

kernel(**inputs) takes the FULL inputs and returns (volume, valid) exactly
like the reference. Internally it shards the voxel grid along nx across 8
NeuronCores, runs a Bass/Tile kernel per core, and concatenates the slabs.

Device pipeline per core (separable-camera fast path):
  stage A: gpsimd ap_gather x-gathers image columns px(gx,gz) from an SBUF
           table [h, (w, c)] (bf16), amortizing the per-index cost over the
           32-channel payload -> A[h, (gz, gx, c)]
  stage B: PE one-hot matmul Sel[h, gy] expands+transposes to
           [(gx, c), gy] per gz plane (PSUM)
  drain:   ACT/DVE copy PSUM -> SBUF accumulator with a gz-strided access
           pattern, building [(gx, c), (gy, gz)] runs
  write:   contiguous 40KB-per-partition DMA to volume[b, c, gx, :, :]
Invalid voxels read a zero row/column of the table (sentinel index), so no
mask multiply is needed. The tiny per-plane index math (2*64*(160+20)
values) runs on host; all O(N)-scale data movement happens on device.
"""

import subprocess

import numpy as np
import ml_dtypes
import orjson

import concourse.bass as bass  # noqa: F401
import concourse.mybir as mybir
from concourse import bacc, bass2jax, bass_utils
from concourse.tile import TileContext

# ---------------------------------------------------------------------------
# Container compile fixups:
#  - the walrus in this image supports ONE sync-wait per instruction; split
#    extras onto same-engine NoOps (program order makes this equivalent)
#  - use the b16 walrus build (knows the custom ISA instructions)
# ---------------------------------------------------------------------------
_split_counter = [0]


def _split_multi_waits(bir: dict) -> int:
    n_split = 0
    for func in bir.get("functions", []):
        for blk in func.get("blocks", []):
            insts = blk.get("instructions")
            if not insts:
                continue
            out = []
            for ins in insts:
                si = ins.get("sync_info")
                waits = si.get("on_wait") if si else None
                if waits and len(waits) > 1:
                    keep = waits[-1]
                    for w in waits[:-1]:
                        _split_counter[0] += 1
                        out.append(
                            {
                                "name": f"antsplitw-{_split_counter[0]}",
                                "opcode": "NoOp",
                                "engine": ins["engine"],
                                "ins": [],
                                "outs": [],
                                "sync_info": {"on_wait": [w], "on_update": []},
                            }
                        )
                    si["on_wait"] = [keep]
                    n_split += 1
                out.append(ins)
            blk["instructions"] = out
    return n_split


if not getattr(bass_utils, "_atlas_patched", False):
    bass_utils._atlas_patched = True
    _orig_compile_bir_kernel = bass_utils.compile_bir_kernel

    def _patched_compile_bir_kernel(bir_json, tmpdir, neff_name="file.neff"):
        bir = orjson.loads(bir_json)
        if _split_multi_waits(bir):
            bir_json = orjson.dumps(bir)
        try:
            return _orig_compile_bir_kernel(bir_json, tmpdir, neff_name)
        except subprocess.CalledProcessError as e:
            out = e.stdout
            if isinstance(out, bytes):
                out = out.decode(errors="replace")
            lines = (out or "").splitlines()
            tail = [l for l in lines if "ERROR" in l or "Too many" in l]
            print("==== WALRUS COMPILE FAILURE ====")
            print("\n".join(tail[-8:]) if tail else "\n".join(lines[-25:]))
            print("================================", flush=True)
            raise

    bass_utils.compile_bir_kernel = _patched_compile_bir_kernel
    bass2jax.compile_bir_kernel = _patched_compile_bir_kernel

    import glob as _glob

    _b16 = _glob.glob(
        "/nix/store/*b16-bazel*/lib/python3.13/site-packages/neuronxcc/"
        "starfish/bin/walrus_driver"
    )
    if _b16:
        bass_utils.get_walrus_driver = lambda: _b16[0]


# ---------------------------------------------------------------------------
# Problem constants (hardcoded per spec)
# ---------------------------------------------------------------------------
NX, NY, NZ = 160, 160, 64
VOXEL_SIZE = np.float32(0.04)
STRIDE = 4.0
B, C, H, W = 2, 32, 120, 160
N_CORES = 8
GXC = NX // N_CORES        # 20 gx per core
NQ, GXQ = 5, 4             # gx chunks per core
NYZ = NY * NZ              # 10240
HP = H + 1                 # 121 rows incl zero-sentinel row
WP = W + 1                 # 161 cols incl zero-sentinel col
TBL_F = WP * C             # 5152 table free elems per h-row
BF16 = mybir.dt.bfloat16
F32 = mybir.dt.float32
I16 = mybir.dt.int16
U8 = mybir.dt.uint8


def _build_nc(reps: int = 1):
    """Build the 8-core SPMD Tile kernel graph (finalized Bacc).

    reps > 1 repeats the whole pipeline into the same outputs; used for
    device-time measurement by marginal wall-clock (the per-call axon I/O
    overhead is identical for any reps).
    """
    nc = bacc.Bacc(
        "TRN2", target_bir_lowering=False, debug=False, num_devices=N_CORES
    )
    table_d = nc.declare_dram_parameter("table", [B, 128, TBL_F], BF16, isOutput=False)
    pxw_d = nc.declare_dram_parameter("pxw", [B, NQ, 128, 16], I16, isOutput=False)
    pyw_d = nc.declare_dram_parameter("pyw", [B, NZ, NY], F32, isOutput=False)
    vx_d = nc.declare_dram_parameter("vx", [B, GXC, NZ], U8, isOutput=False)
    vy_d = nc.declare_dram_parameter("vy", [B, NY, NZ], U8, isOutput=False)
    # single output: channels 0..31 = volume, channel 32 = valid (as f32)
    vol_d = nc.declare_dram_parameter(
        "vol", [B, C + 1, GXC, NY, NZ], F32, isOutput=True
    )

    with TileContext(nc) as tc:
        with (
            tc.tile_pool(name="const", bufs=1) as constp,
            tc.tile_pool(name="selp", bufs=2) as selp,
            tc.tile_pool(name="pybc", bufs=4) as pybc,
            tc.tile_pool(name="idxp", bufs=4) as idxp,
            tc.tile_pool(name="apool", bufs=2) as apool,
            tc.tile_pool(name="accp", bufs=2) as accp,
            tc.tile_pool(name="vldp", bufs=1) as vldp,
            tc.tile_pool(name="psum", bufs=4, space="PSUM") as psump,
        ):
            iota_t = constp.tile([128, 1], F32)
            nc.gpsimd.iota(
                iota_t[:],
                pattern=[[0, 1]],
                base=0,
                channel_multiplier=1,
                allow_small_or_imprecise_dtypes=True,
            )

            tabs = []
            for b in range(B):
                tb = constp.tile([128, TBL_F], BF16, tag="tab")
                nc.sync.dma_start(out=tb[:], in_=table_d[b])
                tabs.append(tb)

            for rep in range(reps):
                sels = []
                for b in range(B):
                    # one-hot Sel[h, gy] per gz, packed [128, 64*160] bf16
                    sel = selp.tile([128, NZ * NY], BF16)
                    for gz in range(NZ):
                        pyt = pybc.tile([128, NY], F32)
                        nc.sync.dma_start(
                            out=pyt[:], in_=pyw_d[b, gz, :].partition_broadcast(128)
                        )
                        nc.vector.tensor_scalar(
                            out=sel[:, gz * NY : (gz + 1) * NY],
                            in0=pyt[:],
                            scalar1=iota_t[:],
                            scalar2=None,
                            op0=mybir.AluOpType.is_equal,
                        )
                    sels.append(sel)

                    # valid[b] = vx[gx, gz] & vy[gy, gz] -> [20, gy, gz] as f32
                    vyt = vldp.tile([GXC, NY, NZ], U8, tag="vy")
                    nc.sync.dma_start(
                        out=vyt[:], in_=vy_d[b].partition_broadcast(GXC)
                    )
                    vxt = vldp.tile([GXC, 1, NZ], U8, tag="vx")
                    nc.sync.dma_start(out=vxt[:, 0, :], in_=vx_d[b])
                    vout = vldp.tile([GXC, NY, NZ], U8, tag="vout")
                    nc.vector.tensor_tensor(
                        out=vout[:],
                        in0=vyt[:],
                        in1=vxt[:].broadcast_to((GXC, NY, NZ)),
                        op=mybir.AluOpType.mult,
                    )
                    nc.gpsimd.dma_start(out=vol_d[b, C], in_=vout[:])

                for b in range(B):
                    for q in range(NQ):
                        it = idxp.tile([128, 16], I16)
                        nc.sync.dma_start(out=it[:], in_=pxw_d[b, q])
                        aq = apool.tile([128, NZ * GXQ * C], BF16)  # [h,(gz,gx,c)]
                        nc.gpsimd.ap_gather(
                            out_ap=aq[:],
                            in_ap=tabs[b][:],
                            idxs_ap=it[:],
                            channels=128,
                            num_elems=WP,
                            d=C,
                            num_idxs=NZ * GXQ,
                        )
                        acc = accp.tile([128, NYZ], BF16)
                        acc3 = acc[:].rearrange("p (gy gz) -> p gy gz", gz=NZ)
                        for i3, g0 in enumerate(range(0, NZ, 3)):
                            n3 = min(3, NZ - g0)
                            ps = psump.tile([128, 480], F32)
                            for j in range(n3):
                                gz = g0 + j
                                nc.tensor.matmul(
                                    ps[:, j * NY : (j + 1) * NY],
                                    aq[:, gz * 128 : (gz + 1) * 128],
                                    sels[b][:, gz * NY : (gz + 1) * NY],
                                    start=True,
                                    stop=True,
                                )
                            src = ps[:, : n3 * NY].rearrange(
                                "p (j gy) -> p j gy", j=n3
                            )
                            dst = acc3[:, :, g0 : g0 + n3].rearrange(
                                "p gy j -> p j gy"
                            )
                            if i3 % 2 == 0:
                                nc.vector.tensor_copy(dst, src)
                            else:
                                nc.scalar.copy(dst, src)
                        nc.gpsimd.dma_start(
                            out=vol_d[b, :C, q * GXQ : (q + 1) * GXQ].rearrange(
                                "c g gy gz -> g c (gy gz)"
                            ),
                            in_=acc[:],
                        )
    nc.finalize()
    return nc


# ---------------------------------------------------------------------------
# Host-side prep: per-plane projection math (exact f32, matching reference)
# ---------------------------------------------------------------------------
def _host_prep(projection, features, origin):
    proj = projection.astype(np.float32) * np.array(
        [1.0 / STRIDE, 1.0 / STRIDE, 1.0], np.float32
    )[None, :, None]
    sep = (
        np.all(proj[:, 0, 1] == 0.0)
        and np.all(proj[:, 1, 0] == 0.0)
        and np.all(proj[:, 2, 0] == 0.0)
        and np.all(proj[:, 2, 1] == 0.0)
    )
    if not sep:
        return None

    # Plane-index math on jax-CPU with the reference's exact op sequence, so
    # exact-tie roundings (e.g. py = 1.25*gy - 40 hitting .5) match bitwise.
    # Coefficients of the "other" grid axis are exactly 0 (separability), so
    # evaluating the einsum on a sub-grid reproduces the same floats.
    import jax
    import jax.numpy as jnp

    with jax.default_device(jax.devices("cpu")[0]):
        ogj = jnp.asarray(origin.astype(np.float32))
        projj = jnp.asarray(proj)

        def cam_grid(axis_n, axis_idx):
            # sub-grid over (axis, gz); other spatial coord pinned to 0
            ga = jnp.arange(axis_n)
            gzv = jnp.arange(NZ)
            a, z = jnp.meshgrid(ga, gzv, indexing="ij")
            zero = jnp.zeros_like(a.ravel())
            cols = [zero, zero, z.ravel()]
            cols[axis_idx] = a.ravel()
            coords = jnp.stack(cols).astype(np.float32)
            world = coords[None] * float(VOXEL_SIZE) + ogj[:, :, None]
            n = axis_n * NZ
            world = jnp.concatenate(
                [jnp.broadcast_to(world, (B, 3, n)), jnp.ones((B, 1, n), np.float32)],
                axis=1,
            )
            camera = jnp.einsum("bij,bjn->bin", projj, world)
            pz = camera[:, 2, :]
            p0 = jnp.round(camera[:, 0, :] / pz)
            p1 = jnp.round(camera[:, 1, :] / pz)
            return (
                np.asarray(p0).reshape(B, axis_n, NZ),
                np.asarray(p1).reshape(B, axis_n, NZ),
                np.asarray(pz).reshape(B, axis_n, NZ),
            )

        pxf, _, pzx = cam_grid(NX, 0)   # [B, gx, gz]
        _, pyf, pzy = cam_grid(NY, 1)   # [B, gy, gz]

    with np.errstate(invalid="ignore"):
        vx = (pxf >= 0) & (pxf < W) & (pzx > 0)          # [B, gx, gz]
        vy = (pyf >= 0) & (pyf < H) & (pzy > 0)          # [B, gy, gz]
    pxg = np.where(vx, pxf, np.float32(W)).astype(np.int16).transpose(0, 2, 1)
    pyg = np.where(vy, pyf, np.float32(H)).astype(np.float32).transpose(0, 2, 1)
    vxg = vx.astype(np.uint8)
    vyg = vy.astype(np.uint8)

    # bf16 feature table [b, h(128 padded), (w(161), c)], sentinels zero
    table = np.zeros((B, 128, WP, C), np.float32)
    for b in range(B):
        table[b, :H, :W, :] = features[b].transpose(1, 2, 0)
    table = table.reshape(B, 128, TBL_F).astype(ml_dtypes.bfloat16)

    in_maps = []
    for core in range(N_CORES):
        s = core * GXC
        pxw = np.empty((B, NQ, 128, 16), np.int16)
        for b in range(B):
            for q in range(NQ):
                flat = pxg[b][:, s + q * GXQ : s + (q + 1) * GXQ].reshape(-1)
                wrapped = np.ascontiguousarray(flat.reshape(16, 16).T)
                pxw[b, q] = np.tile(wrapped, (8, 1))
        in_maps.append(
            {
                "table": table,
                "pxw": pxw,
                "pyw": pyg,
                "vx": np.ascontiguousarray(vxg[:, s : s + GXC, :]),
                "vy": vyg,
            }
        )
    return in_maps


# ---------------------------------------------------------------------------
# Persistent SPMD runner (compile once, rebind inputs per call)
# ---------------------------------------------------------------------------
class _Runner:
    def __init__(self, nc):
        import jax
        from jax.sharding import Mesh, PartitionSpec, NamedSharding
        from jax.experimental.shard_map import shard_map
        from concourse.bass2jax import (
            _bass_exec_p,
            install_neuronx_cc_hook,
            partition_id_tensor,
        )

        install_neuronx_cc_hook()
        self.jax = jax
        partition_name = nc.partition_id_tensor.name if nc.partition_id_tensor else None
        in_names, out_names, out_avals, zero_outs = [], [], [], []
        for alloc in nc.m.functions[0].allocations:
            if not isinstance(alloc, mybir.MemoryLocationSet):
                continue
            name = alloc.memorylocations[0].name
            if alloc.kind == "ExternalInput":
                if name != partition_name:
                    in_names.append(name)
            elif alloc.kind == "ExternalOutput":
                shape = tuple(alloc.tensor_shape)
                dtype = mybir.dt.np(alloc.dtype)
                out_names.append(name)
                out_avals.append(jax.core.ShapedArray(shape, dtype))
                zero_outs.append(np.zeros(shape, dtype))
        n_params, n_outs = len(in_names), len(out_avals)
        all_in = list(in_names) + list(out_names)
        if partition_name is not None:
            all_in.append(partition_name)
        self.in_names, self.out_names, self.out_avals = in_names, out_names, out_avals

        def _body(*args):
            operands = list(args)
            if partition_name is not None:
                operands.append(partition_id_tensor())
            return tuple(
                _bass_exec_p.bind(
                    *operands,
                    out_avals=tuple(out_avals),
                    in_names=tuple(all_in),
                    out_names=tuple(out_names),
                    lowering_input_output_aliases=(),
                    sim_require_finite=True,
                    sim_require_nnan=True,
                    nc=nc,
                )
            )

        devices = jax.devices()[:N_CORES]
        mesh = Mesh(np.asarray(devices), ("core",))
        self._sharding = NamedSharding(mesh, PartitionSpec("core"))
        self._fn = jax.jit(
            shard_map(
                _body,
                mesh=mesh,
                in_specs=(PartitionSpec("core"),) * (n_params + n_outs),
                out_specs=(PartitionSpec("core"),) * n_outs,
                check_rep=False,
            ),
            keep_unused=True,
        )
        self._zero_outs = zero_outs
        self._dev_zeros = [
            jax.device_put(
                np.zeros((N_CORES * z.shape[0], *z.shape[1:]), z.dtype),
                self._sharding,
            )
            for z in zero_outs
        ]

    def set_inputs(self, in_maps):
        self._dev_in = [
            self.jax.device_put(
                np.concatenate(
                    [np.asarray(in_maps[c][k]) for c in range(N_CORES)], axis=0
                ),
                self._sharding,
            )
            for k in self.in_names
        ] + self._dev_zeros

    def run_device(self):
        outs = self._fn(*self._dev_in)
        for o in outs:
            o.block_until_ready()
        return outs

    def run(self):
        outs = self.run_device()
        res = {}
        for i, name in enumerate(self.out_names):
            a = np.asarray(outs[i])
            res[name] = a.reshape(N_CORES, *self.out_avals[i].shape)
        return res


_CACHE = {}


def _get_runner():
    if "runner" not in _CACHE:
        _CACHE["runner"] = _Runner(_build_nc())
    return _CACHE["runner"]


def _reference_fallback(projection, features, origin):
    """Numpy fallback for non-separable projections (not hit by the
    benchmark's camera; kept for robustness)."""
    proj = projection.astype(np.float32) * np.array(
        [1.0 / STRIDE, 1.0 / STRIDE, 1.0], np.float32
    )[None, :, None]
    gx, gy, gz = np.meshgrid(
        np.arange(NX), np.arange(NY), np.arange(NZ), indexing="ij"
    )
    coords = np.stack([gx.ravel(), gy.ravel(), gz.ravel()]).astype(np.float32)
    world = coords[None] * VOXEL_SIZE + origin.astype(np.float32)[:, :, None]
    N = NX * NY * NZ
    world = np.concatenate(
        [np.broadcast_to(world, (B, 3, N)), np.ones((B, 1, N), np.float32)], axis=1
    )
    camera = np.einsum("bij,bjn->bin", proj, world)
    pz = camera[:, 2, :]
    with np.errstate(divide="ignore", invalid="ignore", over="ignore"):
        px = np.rint(camera[:, 0, :] / pz)
        py = np.rint(camera[:, 1, :] / pz)
        valid = (px >= 0) & (py >= 0) & (px < W) & (py < H) & (pz > 0)
        pxi = np.clip(px, 0, W - 1).astype(np.int32)
        pyi = np.clip(py, 0, H - 1).astype(np.int32)
    idx = pyi * W + pxi
    feat_flat = features.reshape(B, C, H * W)
    vol = np.take_along_axis(feat_flat, idx[:, None, :], axis=2)
    vol = vol * valid[:, None, :].astype(features.dtype)
    return (
        vol.reshape(B, C, NX, NY, NZ),
        valid.reshape(B, 1, NX, NY, NZ),
    )


def kernel(projection, features, origin):
    projection = np.asarray(projection)
    features = np.asarray(features, dtype=np.float32)
    origin = np.asarray(origin)
    in_maps = _host_prep(projection, features, origin)
    if in_maps is None:
        return _reference_fallback(projection, features, origin)
    r = _get_runner()
    r.set_inputs(in_maps)
    res = r.run()
    out = np.concatenate([res["vol"][c] for c in range(N_CORES)], axis=2)
    vol = np.ascontiguousarray(out[:, :C])
    valid = out[:, C] != 0
    return vol, valid.reshape(B, 1, NX, NY, NZ)


# revision 15
# speedup vs baseline: 4646.2965x; 1695.7558x over previous
"""Trainium2 Bass kernel for nn_AtlasDetection (voxel->pixel feature gather).

kernel(**inputs) takes the FULL inputs and returns (volume, valid) exactly
like the reference. Internally it shards the voxel grid along nx across 8
NeuronCores, runs a Bass/Tile kernel per core, and concatenates the slabs.

Device pipeline per core (separable-camera fast path):
  stage A: gpsimd ap_gather x-gathers image columns px(gx,gz) from an SBUF
           table [h, (w, c)] (bf16), amortizing the per-index cost over the
           32-channel payload -> A[h, (gz, gx, c)]
  stage B: PE one-hot matmul Sel[h, gy] expands+transposes to
           [(gx, c), gy] per gz plane (PSUM)
  drain:   ACT/DVE copy PSUM -> SBUF accumulator with a gz-strided access
           pattern, building [(gx, c), (gy, gz)] runs
  write:   contiguous 40KB-per-partition DMA to volume[b, c, gx, :, :]
Invalid voxels read a zero row/column of the table (sentinel index), so no
mask multiply is needed. The tiny per-plane index math (2*64*(160+20)
values) runs on host; all O(N)-scale data movement happens on device.
"""

import subprocess

import numpy as np
import ml_dtypes
import orjson

import concourse.bass as bass  # noqa: F401
import concourse.mybir as mybir
from concourse import bacc, bass2jax, bass_utils
from concourse.tile import TileContext

# ---------------------------------------------------------------------------
# Container compile fixups:
#  - the walrus in this image supports ONE sync-wait per instruction; split
#    extras onto same-engine NoOps (program order makes this equivalent)
#  - use the b16 walrus build (knows the custom ISA instructions)
# ---------------------------------------------------------------------------
_split_counter = [0]


def _split_multi_waits(bir: dict) -> int:
    n_split = 0
    for func in bir.get("functions", []):
        for blk in func.get("blocks", []):
            insts = blk.get("instructions")
            if not insts:
                continue
            out = []
            for ins in insts:
                si = ins.get("sync_info")
                waits = si.get("on_wait") if si else None
                if waits and len(waits) > 1:
                    keep = waits[-1]
                    for w in waits[:-1]:
                        _split_counter[0] += 1
                        out.append(
                            {
                                "name": f"antsplitw-{_split_counter[0]}",
                                "opcode": "NoOp",
                                "engine": ins["engine"],
                                "ins": [],
                                "outs": [],
                                "sync_info": {"on_wait": [w], "on_update": []},
                            }
                        )
                    si["on_wait"] = [keep]
                    n_split += 1
                out.append(ins)
            blk["instructions"] = out
    return n_split


if not getattr(bass_utils, "_atlas_patched", False):
    bass_utils._atlas_patched = True
    _orig_compile_bir_kernel = bass_utils.compile_bir_kernel

    def _patched_compile_bir_kernel(bir_json, tmpdir, neff_name="file.neff"):
        bir = orjson.loads(bir_json)
        if _split_multi_waits(bir):
            bir_json = orjson.dumps(bir)
        try:
            return _orig_compile_bir_kernel(bir_json, tmpdir, neff_name)
        except subprocess.CalledProcessError as e:
            out = e.stdout
            if isinstance(out, bytes):
                out = out.decode(errors="replace")
            lines = (out or "").splitlines()
            tail = [l for l in lines if "ERROR" in l or "Too many" in l]
            print("==== WALRUS COMPILE FAILURE ====")
            print("\n".join(tail[-8:]) if tail else "\n".join(lines[-25:]))
            print("================================", flush=True)
            raise

    bass_utils.compile_bir_kernel = _patched_compile_bir_kernel
    bass2jax.compile_bir_kernel = _patched_compile_bir_kernel

    import glob as _glob

    _b16 = _glob.glob(
        "/nix/store/*b16-bazel*/lib/python3.13/site-packages/neuronxcc/"
        "starfish/bin/walrus_driver"
    )
    if _b16:
        bass_utils.get_walrus_driver = lambda: _b16[0]


# ---------------------------------------------------------------------------
# Problem constants (hardcoded per spec)
# ---------------------------------------------------------------------------
NX, NY, NZ = 160, 160, 64
VOXEL_SIZE = np.float32(0.04)
STRIDE = 4.0
B, C, H, W = 2, 32, 120, 160
N_CORES = 8
GXC = NX // N_CORES        # 20 gx per core
NQ, GXQ = 5, 4             # gx chunks per core
NYZ = NY * NZ              # 10240
HP = H + 1                 # 121 rows incl zero-sentinel row
WP = W + 1                 # 161 cols incl zero-sentinel col
TBL_F = WP * C             # 5152 table free elems per h-row
BF16 = mybir.dt.bfloat16
F32 = mybir.dt.float32
I16 = mybir.dt.int16
U8 = mybir.dt.uint8


def _build_nc(reps: int = 1, packed: bool = True):
    """Build the 8-core SPMD Tile kernel graph (finalized Bacc).

    packed=True gathers both batches' channels in one 64-element payload
    per pixel index (requires identical px grids across the batch, true
    whenever the two projections are equal). packed=False is the general
    per-batch path.

    reps > 1 repeats the whole pipeline into the same outputs; used for
    device-time measurement by marginal wall-clock (the per-call axon I/O
    overhead is identical for any reps).
    """
    nc = bacc.Bacc(
        "TRN2", target_bir_lowering=False, debug=False, num_devices=N_CORES
    )
    table_d = nc.declare_dram_parameter("table", [128, B * TBL_F], BF16, isOutput=False)
    pxw_d = nc.declare_dram_parameter("pxw", [B, NQ, 128, 16], I16, isOutput=False)
    pyw_d = nc.declare_dram_parameter("pyw", [B, NZ, NY], F32, isOutput=False)
    vx_d = nc.declare_dram_parameter("vx", [B, GXC, NZ], U8, isOutput=False)
    vy_d = nc.declare_dram_parameter("vy", [B, NY, NZ], U8, isOutput=False)
    # single output: channels 0..31 = volume, channel 32 = valid (as f32)
    vol_d = nc.declare_dram_parameter(
        "vol", [B, C + 1, GXC, NY, NZ], F32, isOutput=True
    )

    for rep in range(reps):
        with TileContext(nc) as tc, (
            __import__("contextlib").ExitStack()
        ) as _stk:
            constp = _stk.enter_context(tc.tile_pool(name="const", bufs=1))
            selp = _stk.enter_context(tc.tile_pool(name="selp", bufs=2))
            pybc = _stk.enter_context(tc.tile_pool(name="pybc", bufs=4))
            idxp = _stk.enter_context(tc.tile_pool(name="idxp", bufs=4))
            apool = _stk.enter_context(tc.tile_pool(name="apool", bufs=2))
            accp = _stk.enter_context(tc.tile_pool(name="accp", bufs=2))
            vldp = _stk.enter_context(tc.tile_pool(name="vldp", bufs=1))
            psump = _stk.enter_context(
                tc.tile_pool(name="psum", bufs=4, space="PSUM")
            )
            iota_t = constp.tile([128, 1], F32)
            nc.gpsimd.iota(
                iota_t[:],
                pattern=[[0, 1]],
                base=0,
                channel_multiplier=1,
                allow_small_or_imprecise_dtypes=True,
            )
            tab = constp.tile([128, B * TBL_F], BF16)
            nc.sync.dma_start(out=tab[:], in_=table_d[:])

            sels = []
            n_sel = 1 if packed else B
            for b in range(n_sel):
                # one-hot Sel[h, gy] per gz, packed [128, 64*160] bf16
                sel = selp.tile([128, NZ * NY], BF16)
                for gz in range(NZ):
                    pyt = pybc.tile([128, NY], F32)
                    nc.sync.dma_start(
                        out=pyt[:], in_=pyw_d[b, gz, :].partition_broadcast(128)
                    )
                    nc.vector.tensor_scalar(
                        out=sel[:, gz * NY : (gz + 1) * NY],
                        in0=pyt[:],
                        scalar1=iota_t[:],
                        scalar2=None,
                        op0=mybir.AluOpType.is_equal,
                    )
                sels.append(sel)

            for b in range(B):
                # valid[b] = vx[gx, gz] & vy[gy, gz] -> [20, gy, gz] (in place)
                vyt = vldp.tile([GXC, NY, NZ], U8, tag="vy")
                nc.sync.dma_start(out=vyt[:], in_=vy_d[b].partition_broadcast(GXC))
                vxt = vldp.tile([GXC, 1, NZ], U8, tag="vx")
                nc.sync.dma_start(out=vxt[:, 0, :], in_=vx_d[b])
                nc.vector.tensor_tensor(
                    out=vyt[:],
                    in0=vyt[:],
                    in1=vxt[:].broadcast_to((GXC, NY, NZ)),
                    op=mybir.AluOpType.mult,
                )
                nc.gpsimd.dma_start(out=vol_d[b, C], in_=vyt[:])

            def stage_b(sel, out_dmas, gz_to_lhsT):
                acc = accp.tile([128, NYZ], BF16)
                acc3 = acc[:].rearrange("p (gy gz) -> p gy gz", gz=NZ)
                for i3, g0 in enumerate(range(0, NZ, 3)):
                    n3 = min(3, NZ - g0)
                    ps = psump.tile([128, 480], F32)
                    for j in range(n3):
                        gz = g0 + j
                        nc.tensor.matmul(
                            ps[:, j * NY : (j + 1) * NY],
                            gz_to_lhsT(gz),
                            sel[:, gz * NY : (gz + 1) * NY],
                            start=True,
                            stop=True,
                        )
                    src = ps[:, : n3 * NY].rearrange("p (j gy) -> p j gy", j=n3)
                    dst = acc3[:, :, g0 : g0 + n3].rearrange("p gy j -> p j gy")
                    if i3 % 2 == 0:
                        nc.vector.tensor_copy(dst, src)
                    else:
                        nc.scalar.copy(dst, src)
                for out_ap, in_sel in out_dmas:
                    nc.gpsimd.dma_start(out=out_ap, in_=in_sel(acc))

            if packed:
                # one gather per q with 64-wide (b, c) payload; each matmul
                # covers a contiguous (2gx, 2b, 32c) = 128-column block and
                # shares the single Sel (py grids equal across batch).
                for q in range(NQ):
                    it = idxp.tile([128, 16], I16)
                    nc.sync.dma_start(out=it[:], in_=pxw_d[0, q])
                    aq = apool.tile([128, NZ * GXQ * B * C], BF16)
                    nc.gpsimd.ap_gather(
                        out_ap=aq[:],
                        in_ap=tab[:],
                        idxs_ap=it[:],
                        channels=128,
                        num_elems=WP,
                        d=B * C,
                        num_idxs=NZ * GXQ,
                    )
                    for gxp in range(2):
                        g0c = q * GXQ + gxp * 2
                        # one DMA per batch: acc partitions (g2, b2, c32)
                        out_dmas = [
                            (
                                vol_d[b, :C, g0c : g0c + 2].rearrange(
                                    "c g gy gz -> g c (gy gz)"
                                ),
                                lambda acc, b=b: acc[:].rearrange(
                                    "(g b c) f -> g b c f", g=2, b=B
                                )[:, b],
                            )
                            for b in range(B)
                        ]
                        stage_b(
                            sels[0], out_dmas,
                            lambda gz, gxp=gxp: aq[
                                :, gz * 256 + gxp * 128 : gz * 256 + (gxp + 1) * 128
                            ],
                        )
            else:
                for b in range(B):
                    for q in range(NQ):
                        it = idxp.tile([128, 16], I16)
                        nc.sync.dma_start(out=it[:], in_=pxw_d[b, q])
                        aq = apool.tile([128, NZ * GXQ * C], BF16)
                        nc.gpsimd.ap_gather(
                            out_ap=aq[:],
                            in_ap=tab[:, b * TBL_F : (b + 1) * TBL_F],
                            idxs_ap=it[:],
                            channels=128,
                            num_elems=WP,
                            d=C,
                            num_idxs=NZ * GXQ,
                        )
                        out_ap = vol_d[b, :C, q * GXQ : (q + 1) * GXQ].rearrange(
                            "c g gy gz -> g c (gy gz)"
                        )
                        stage_b(
                            sels[b], [(out_ap, lambda acc: acc[:])],
                            lambda gz: aq[:, gz * 128 : (gz + 1) * 128],
                        )
    nc.finalize()
    return nc


# ---------------------------------------------------------------------------
# Host-side prep: per-plane projection math (exact f32, matching reference)
# ---------------------------------------------------------------------------
def _host_prep(projection, features, origin):
    proj = projection.astype(np.float32) * np.array(
        [1.0 / STRIDE, 1.0 / STRIDE, 1.0], np.float32
    )[None, :, None]
    sep = (
        np.all(proj[:, 0, 1] == 0.0)
        and np.all(proj[:, 1, 0] == 0.0)
        and np.all(proj[:, 2, 0] == 0.0)
        and np.all(proj[:, 2, 1] == 0.0)
    )
    if not sep:
        return None

    # Plane-index math on jax-CPU with the reference's exact op sequence, so
    # exact-tie roundings (e.g. py = 1.25*gy - 40 hitting .5) match bitwise.
    # Coefficients of the "other" grid axis are exactly 0 (separability), so
    # evaluating the einsum on a sub-grid reproduces the same floats.
    import jax
    import jax.numpy as jnp

    with jax.default_device(jax.devices("cpu")[0]):
        ogj = jnp.asarray(origin.astype(np.float32))
        projj = jnp.asarray(proj)

        def cam_grid(axis_n, axis_idx):
            # sub-grid over (axis, gz); other spatial coord pinned to 0
            ga = jnp.arange(axis_n)
            gzv = jnp.arange(NZ)
            a, z = jnp.meshgrid(ga, gzv, indexing="ij")
            zero = jnp.zeros_like(a.ravel())
            cols = [zero, zero, z.ravel()]
            cols[axis_idx] = a.ravel()
            coords = jnp.stack(cols).astype(np.float32)
            world = coords[None] * float(VOXEL_SIZE) + ogj[:, :, None]
            n = axis_n * NZ
            world = jnp.concatenate(
                [jnp.broadcast_to(world, (B, 3, n)), jnp.ones((B, 1, n), np.float32)],
                axis=1,
            )
            camera = jnp.einsum("bij,bjn->bin", projj, world)
            pz = camera[:, 2, :]
            p0 = jnp.round(camera[:, 0, :] / pz)
            p1 = jnp.round(camera[:, 1, :] / pz)
            return (
                np.asarray(p0).reshape(B, axis_n, NZ),
                np.asarray(p1).reshape(B, axis_n, NZ),
                np.asarray(pz).reshape(B, axis_n, NZ),
            )

        pxf, _, pzx = cam_grid(NX, 0)   # [B, gx, gz]
        _, pyf, pzy = cam_grid(NY, 1)   # [B, gy, gz]

    with np.errstate(invalid="ignore"):
        vx = (pxf >= 0) & (pxf < W) & (pzx > 0)          # [B, gx, gz]
        vy = (pyf >= 0) & (pyf < H) & (pzy > 0)          # [B, gy, gz]
    pxg = np.where(vx, pxf, np.float32(W)).astype(np.int16).transpose(0, 2, 1)
    pyg = np.where(vy, pyf, np.float32(H)).astype(np.float32).transpose(0, 2, 1)
    vxg = vx.astype(np.uint8)
    vyg = vy.astype(np.uint8)

    # packed mode (single 64-wide gather payload for both batches) is
    # implemented but its acc-partition mapping is not yet validated on
    # hardware; ship the proven per-batch path.
    packed = False

    # bf16 feature table, zero sentinels. packed: [h, (w, b, c)];
    # unpacked: [h, (b, w, c)] so tab[:, b*TBL_F:(b+1)*TBL_F] is batch b.
    if packed:
        table = np.zeros((128, WP, B, C), np.float32)
        for b in range(B):
            table[:H, :W, b, :] = features[b].transpose(1, 2, 0)
    else:
        table = np.zeros((128, B, WP, C), np.float32)
        for b in range(B):
            table[:H, b, :W, :] = features[b].transpose(1, 2, 0)
    table = table.reshape(128, B * TBL_F).astype(ml_dtypes.bfloat16)

    in_maps = []
    for core in range(N_CORES):
        s = core * GXC
        pxw = np.empty((B, NQ, 128, 16), np.int16)
        for b in range(B):
            for q in range(NQ):
                flat = pxg[b][:, s + q * GXQ : s + (q + 1) * GXQ].reshape(-1)
                wrapped = np.ascontiguousarray(flat.reshape(16, 16).T)
                pxw[b, q] = np.tile(wrapped, (8, 1))
        in_maps.append(
            {
                "table": table,
                "pxw": pxw,
                "pyw": pyg,
                "vx": np.ascontiguousarray(vxg[:, s : s + GXC, :]),
                "vy": vyg,
            }
        )
    return in_maps, packed


# ---------------------------------------------------------------------------
# Persistent SPMD runner (compile once, rebind inputs per call)
# ---------------------------------------------------------------------------
class _Runner:
    def __init__(self, nc):
        import jax
        from jax.sharding import Mesh, PartitionSpec, NamedSharding
        from jax.experimental.shard_map import shard_map
        from concourse.bass2jax import (
            _bass_exec_p,
            install_neuronx_cc_hook,
            partition_id_tensor,
        )

        install_neuronx_cc_hook()
        self.jax = jax
        partition_name = nc.partition_id_tensor.name if nc.partition_id_tensor else None
        in_names, out_names, out_avals, zero_outs = [], [], [], []
        for alloc in nc.m.functions[0].allocations:
            if not isinstance(alloc, mybir.MemoryLocationSet):
                continue
            name = alloc.memorylocations[0].name
            if alloc.kind == "ExternalInput":
                if name != partition_name:
                    in_names.append(name)
            elif alloc.kind == "ExternalOutput":
                shape = tuple(alloc.tensor_shape)
                dtype = mybir.dt.np(alloc.dtype)
                out_names.append(name)
                out_avals.append(jax.core.ShapedArray(shape, dtype))
                zero_outs.append(np.zeros(shape, dtype))
        n_params, n_outs = len(in_names), len(out_avals)
        all_in = list(in_names) + list(out_names)
        if partition_name is not None:
            all_in.append(partition_name)
        self.in_names, self.out_names, self.out_avals = in_names, out_names, out_avals

        def _body(*args):
            operands = list(args)
            if partition_name is not None:
                operands.append(partition_id_tensor())
            return tuple(
                _bass_exec_p.bind(
                    *operands,
                    out_avals=tuple(out_avals),
                    in_names=tuple(all_in),
                    out_names=tuple(out_names),
                    lowering_input_output_aliases=(),
                    sim_require_finite=True,
                    sim_require_nnan=True,
                    nc=nc,
                )
            )

        devices = jax.devices()[:N_CORES]
        mesh = Mesh(np.asarray(devices), ("core",))
        self._sharding = NamedSharding(mesh, PartitionSpec("core"))
        self._fn = jax.jit(
            shard_map(
                _body,
                mesh=mesh,
                in_specs=(PartitionSpec("core"),) * (n_params + n_outs),
                out_specs=(PartitionSpec("core"),) * n_outs,
                check_rep=False,
            ),
            keep_unused=True,
        )
        self._zero_outs = zero_outs
        self._dev_zeros = [
            jax.device_put(
                np.zeros((N_CORES * z.shape[0], *z.shape[1:]), z.dtype),
                self._sharding,
            )
            for z in zero_outs
        ]

    def set_inputs(self, in_maps):
        self._dev_in = [
            self.jax.device_put(
                np.concatenate(
                    [np.asarray(in_maps[c][k]) for c in range(N_CORES)], axis=0
                ),
                self._sharding,
            )
            for k in self.in_names
        ] + self._dev_zeros

    def run_device(self):
        outs = self._fn(*self._dev_in)
        for o in outs:
            o.block_until_ready()
        return outs

    def run(self):
        outs = self.run_device()
        res = {}
        for i, name in enumerate(self.out_names):
            a = np.asarray(outs[i])
            res[name] = a.reshape(N_CORES, *self.out_avals[i].shape)
        return res


_CACHE = {}


def _get_runner(packed: bool):
    key = ("runner", packed)
    if key not in _CACHE:
        _CACHE[key] = _Runner(_build_nc(packed=packed))
    return _CACHE[key]


def _reference_fallback(projection, features, origin):
    """Numpy fallback for non-separable projections (not hit by the
    benchmark's camera; kept for robustness)."""
    proj = projection.astype(np.float32) * np.array(
        [1.0 / STRIDE, 1.0 / STRIDE, 1.0], np.float32
    )[None, :, None]
    gx, gy, gz = np.meshgrid(
        np.arange(NX), np.arange(NY), np.arange(NZ), indexing="ij"
    )
    coords = np.stack([gx.ravel(), gy.ravel(), gz.ravel()]).astype(np.float32)
    world = coords[None] * VOXEL_SIZE + origin.astype(np.float32)[:, :, None]
    N = NX * NY * NZ
    world = np.concatenate(
        [np.broadcast_to(world, (B, 3, N)), np.ones((B, 1, N), np.float32)], axis=1
    )
    camera = np.einsum("bij,bjn->bin", proj, world)
    pz = camera[:, 2, :]
    with np.errstate(divide="ignore", invalid="ignore", over="ignore"):
        px = np.rint(camera[:, 0, :] / pz)
        py = np.rint(camera[:, 1, :] / pz)
        valid = (px >= 0) & (py >= 0) & (px < W) & (py < H) & (pz > 0)
        pxi = np.clip(px, 0, W - 1).astype(np.int32)
        pyi = np.clip(py, 0, H - 1).astype(np.int32)
    idx = pyi * W + pxi
    feat_flat = features.reshape(B, C, H * W)
    vol = np.take_along_axis(feat_flat, idx[:, None, :], axis=2)
    vol = vol * valid[:, None, :].astype(features.dtype)
    return (
        vol.reshape(B, C, NX, NY, NZ),
        valid.reshape(B, 1, NX, NY, NZ),
    )


def kernel(projection, features, origin):
    projection = np.asarray(projection)
    features = np.asarray(features, dtype=np.float32)
    origin = np.asarray(origin)
    prep = _host_prep(projection, features, origin)
    if prep is None:
        return _reference_fallback(projection, features, origin)
    in_maps, packed = prep
    r = _get_runner(packed)
    r.set_inputs(in_maps)
    res = r.run()
    out = np.concatenate([res["vol"][c] for c in range(N_CORES)], axis=2)
    vol = np.ascontiguousarray(out[:, :C])
    valid = out[:, C] != 0
    return vol, valid.reshape(B, 1, NX, NY, NZ)
